# revision 53
# baseline (speedup 1.0000x reference)
"""MiniBatchDiscrimination kernel for 8 Trainium2 NeuronCores.

Math: m = (x @ T).reshape(B, K, D); l1[i,k,j] = sum_d |m[i,k,d]-m[j,k,d]|;
feat[i,k] = sum_j exp(-l1[i,k,j]); out = concat([x, feat], axis=1).

Sharding: data-parallel over i-rows (128 per core). Every core computes the
full projection m.T on its own PE (redundant but cheap, avoids collectives),
plus m restricted to its own 128 rows, then evaluates its [128, K, B] slice
of the pairwise kernel.

All matmuls run fp16 (4x PE throughput vs fp32); feat errors are ~1e-2
absolute where the tolerance is ~1e-1 (feat = 1 + tiny cross terms).

Per (k, slot) the PE produces diff[i, j] = mT[kd, j] - m_mine[i, kd] into
PSUM as one contraction-2 fp16 matmul: lhsT = [ones_row; m_mine_col],
rhs = [mT_row; -ones_row]. PE operands must start at partition 0, so the
per-kd rows are staged to partitions 0/1 of small SBUF staging tiles by DMA
on the SP queue (groups of 8 kd, double-buffered, prefetched).

The |.| + d-sum stage is spread over three engines. This walrus build only
lowers abs on Act activations and on DVE TensorReduce (abs_max / max / min
with a PSUM operand fail the BIR verifier), so per k:
  r01 = |d0|+|d1|  two [128,2,512] PSUM tiles -> DVE reduce-abs-add over
                   the d-axis (one op per j-half, fuses abs AND the add)
  r23 = |d2|+|d3|  two [128,2,512] PSUM tiles -> Act flat Abs to fp16,
                   halves summed on Pool
  wa  = |d4|       [128,2,512] PSUM tile -> Act Abs (jh0) + DVE reduce (jh1)
  l1  = r01+wa+r23 adds split DVE/Pool
  feat[:,k] = accum_out(exp(-l1))  on Act
Emission order C0,C1,A0,A1,W minimizes PSUM-ring stalls (5 tiles rotate
through 4 pool slots; a tile's slot frees only when its consumer retires).
"""

import numpy as np
from contextlib import ExitStack

import concourse.bass as bass
import concourse.tile as tile
from concourse import mybir
from concourse.bass_utils import run_bass_kernel_spmd
from concourse.masks import make_identity

B, F = 1024, 1024
K, D = 100, 5
KD = K * D            # 500
NCORES = 8
RPC = B // NCORES     # 128 i-rows per core
KDB = 128             # kd rows per projection block
NKDB = (KD + KDB - 1) // KDB   # 4 (last block 116 rows)
G = 16                # kd entries per staging refill group
NG = (KD + G - 1) // G         # 32
FP32 = mybir.dt.float32
FP16 = mybir.dt.float16
AF = mybir.ActivationFunctionType
ALU = mybir.AluOpType


class TC(tile.TileContext):
    """TileContext whose tail puts sem waits on NOPs instead of the Drain.

    The walrus in this container lowers Drain/NOP with a no-sync-struct ISA
    encoding that holds at most one wait, so the stock tail drain (which
    carries one wait per outstanding proc) fails codegen. Emit one NOP per
    proc, each carrying a single wait, before the drain.
    """

    def _drain_and_barrier(self, tick_clock, wait_clock):
        from concourse.vector_clock import ScopedClock, VectorClock

        gc = tick_clock.global_clock
        n = len(gc)
        for p in range(n):
            t = gc[p]
            if t <= 0:
                continue
            vec = [0] * n
            vec[p] = t
            nop_inst = self.nc.sync.nop(nofuse=True)
            wait_clock.add_sem_waits(
                nop_inst.ins, ScopedClock({None: VectorClock(vec)})
            )
        self.nc.sync.drain()
        self.nc.all_engine_barrier()
        popped = self.nc._tile_sem_poison_stack.pop()
        assert popped is self._sem_poison
        self.nc.clear_and_free_semaphores(list(self.sems.allocated().values()))
        self.nc.all_engine_barrier()


def _hoist_excess_waits(nc):
    """Move excess sem waits onto same-engine NOPs inserted just before.

    This container's walrus encodes Matmult (LDWEIGHTS struct) and
    NoOp/Drain with room for a single sync wait; Tile may attach several.
    Keep one wait on the instruction and carry the rest on dedicated NOPs,
    which is sync-equivalent (same engine, program order).
    """
    def limit_for(inst):
        return 1
    for f in nc.m.functions:
        for bb in f.blocks:
            snapshot = list(bb.instructions)
            if not any(
                i.sync_info is not None
                and len(i.sync_info.on_wait) > limit_for(i)
                for i in snapshot
            ):
                continue
            new_list = []
            for inst in snapshot:
                lim = limit_for(inst)
                si = inst.sync_info
                if lim is not None and si is not None and \
                        len(si.on_wait) > lim:
                    waits = list(si.on_wait)
                    for w in waits[:-lim]:
                        bi = nc.engines[inst.engine].nop(nofuse=True)
                        found = False
                        for f2 in nc.m.functions:
                            for bb2 in f2.blocks:
                                tail = bb2.instructions
                                if tail and tail[-1].name == bi.ins.name:
                                    tail.pop()
                                    found = True
                                    break
                            if found:
                                break
                        assert found, bi.ins.name
                        bi.ins.sync_info = mybir.SyncInfo(
                            on_wait=[w], on_update=[])
                        new_list.append(bi.ins)
                    inst.sync_info = mybir.SyncInfo(
                        on_wait=waits[-lim:], on_update=list(si.on_update))
                new_list.append(inst)
            bb.instructions = new_list


# main-loop emission order of the 5 PSUM tiles per k (sweepable):
# A0/A1 = (d0,d1) pair-halves -> DVE reduce; C0/C1 = (d2,d3) -> Act
# flat-abs + Pool merge; W = d4 -> Act half + DVE reduce half.
V_EMIT = ["C0", "C1", "A0", "A1", "W"]
V_DEPTH = 1   # tail pipeline depth (k - V_DEPTH)


def build_nc(reps: int = 1):
    nc = bass.Bass()
    x_d = nc.dram_tensor("x", [B, F], FP32, kind="ExternalInput")
    t_d = nc.dram_tensor("t", [F, KD], FP32, kind="ExternalInput")
    xm_d = nc.dram_tensor("xm", [RPC, F], FP32, kind="ExternalInput")
    out_d = nc.dram_tensor("out", [RPC, F + K], FP32, kind="ExternalOutput")

    with TC(nc) as tc, ExitStack() as ctx, \
            nc.allow_low_precision(reason="l1 accumulated in fp16; feat "
                                   "tolerance is ~1e-1 absolute"):
        const = ctx.enter_context(tc.tile_pool(name="const", bufs=1))

        feat = const.tile([RPC, K], FP32, tag="feat")
        # fp16 transformed projection: mT[b] = (x @ T').T block [kdn, B]
        # and this core's rows mTm[b] = (xm @ T').T block [kdn, RPC]
        mT = [const.tile([KDB, B], FP16, tag=f"mT{b}", name=f"mT{b}")
              for b in range(NKDB)]
        mTm = [const.tile([KDB, RPC], FP16, tag=f"mTm{b}", name=f"mTm{b}")
               for b in range(NKDB)]

        with ExitStack() as setup_ctx:
            setup_sb = setup_ctx.enter_context(
                tc.tile_pool(name="setup_sb", bufs=3))
            tpsum = setup_ctx.enter_context(
                tc.tile_pool(name="tpsum", bufs=4, space="PSUM"))
            gpsum = setup_ctx.enter_context(
                tc.tile_pool(name="gpsum", bufs=2, space="PSUM"))
            scon = setup_ctx.enter_context(
                tc.tile_pool(name="scon", bufs=1))

            ident = scon.tile([128, 128], FP16, tag="ident")
            make_identity(nc, ident)

            # T in 8 f-blocks [128, KD] fp32, converted to fp16
            tp16 = []
            for fb in range(8):
                tt = setup_sb.tile([128, KD], FP32, tag="ttin", name="ttin")
                (nc.sync if fb % 2 else nc.gpsimd).dma_start(
                    tt[:], t_d[fb * 128:(fb + 1) * 128, :])
                tp = scon.tile([128, KD], FP16, tag=f"tp{fb}", name=f"tp{fb}")
                nc.vector.tensor_copy(tp[:], tt[:])
                tp16.append(tp)

            # x transposed into 8 f-block fp16 tiles xT[fb] = [128(f), B(j)]
            xT = [scon.tile([128, B], FP16, tag=f"xT{fb}", name=f"xT{fb}")
                  for fb in range(8)]
            for jb in range(8):
                xin = setup_sb.tile([128, F], FP32, tag="xin")
                (nc.sync if jb % 2 else nc.gpsimd).dma_start(
                    xin[:], x_d[jb * 128:(jb + 1) * 128, :])
                x16 = setup_sb.tile([128, F], FP16, tag="x16", name="x16")
                nc.vector.tensor_copy(x16[:], xin[:])
                for fb in range(8):
                    ps = tpsum.tile([128, 128], FP16, tag="tps")
                    nc.tensor.transpose(
                        ps[:], x16[:, fb * 128:(fb + 1) * 128], ident[:])
                    dst = xT[fb][:, jb * 128:(jb + 1) * 128]
                    if fb % 2:
                        nc.scalar.activation(dst, ps[:], AF.Copy)
                    else:
                        nc.vector.tensor_copy(dst, ps[:])

            # this core's rows: load, emit x-part of the output, transpose
            xmT = scon.tile([128, F], FP16, tag="xmT")
            xm_in = setup_sb.tile([RPC, F], FP32, tag="xmin")
            nc.sync.dma_start(xm_in[:], xm_d[:])
            nc.sync.dma_start(out_d[:, 0:F], xm_in[:])
            xm16 = scon.tile([RPC, F], FP16, tag="xm16")
            nc.vector.tensor_copy(xm16[:], xm_in[:])
            for fb in range(8):
                ps = tpsum.tile([128, 128], FP16, tag="tps")
                nc.tensor.transpose(
                    ps[:], xm16[:, fb * 128:(fb + 1) * 128], ident[:])
                dst = xmT[:, fb * 128:(fb + 1) * 128]
                if fb % 2:
                    nc.scalar.activation(dst, ps[:], AF.Copy)
                else:
                    nc.vector.tensor_copy(dst, ps[:])

            # mT[b] = (x @ T').T block  [kdn, B]  (fp16 GEMM)
            # mTm[b] = (xm @ T').T block [kdn, RPC]
            for b in range(NKDB):
                kd0 = b * KDB
                kdn = min(KDB, KD - kd0)
                for jh in range(2):
                    ps = gpsum.tile([KDB, 512], FP32, tag="gps")
                    for fb in range(8):
                        nc.tensor.matmul(
                            ps[:kdn, :],
                            tp16[fb][:, kd0:kd0 + kdn],
                            xT[fb][:, jh * 512:(jh + 1) * 512],
                            start=(fb == 0), stop=(fb == 7))
                    dst = mT[b][0:kdn, jh * 512:(jh + 1) * 512]
                    if jh:
                        nc.scalar.activation(dst, ps[:kdn, :], AF.Copy)
                    else:
                        nc.vector.tensor_copy(dst, ps[:kdn, :])
                ps2 = gpsum.tile([KDB, RPC], FP32, tag="gps2")
                for fb in range(8):
                    nc.tensor.matmul(
                        ps2[:kdn, :],
                        tp16[fb][:, kd0:kd0 + kdn],
                        xmT[:, fb * 128:(fb + 1) * 128],
                        start=(fb == 0), stop=(fb == 7))
                nc.scalar.activation(mTm[b][0:kdn, :], ps2[:kdn, :],
                                     AF.Copy)

        # ---- main loop over kernels k ----
        sl_tiles = [const.tile([2, G * 128], FP16, tag=f"sl{i}",
                               name=f"sl{i}") for i in range(2)]
        sr_tiles = [const.tile([2, G * 1024], FP16, tag=f"sr{i}",
                               name=f"sr{i}") for i in range(2)]
        # Only the constant rows need init (the data rows are DMA-filled
        # per staging group): sl p0 = ones; sr p1 = -ones, stamped by DMA
        # from a small constant row instead of a 8K-element engine memset.
        negones = const.tile([1, 1024], FP16, tag="negones")
        nc.vector.memset(negones[0:1, :], -1.0)
        for i in range(2):
            nc.vector.memset(sl_tiles[i][0:1, :], 1.0)
            for c in range(G):
                nc.sync.dma_start(
                    sr_tiles[i][1:2, c * 1024:(c + 1) * 1024],
                    negones[0:1, :])
        t_pool = ctx.enter_context(tc.tile_pool(name="tp", bufs=10))
        w_pool = ctx.enter_context(tc.tile_pool(name="wp", bufs=6))
        a_pool = ctx.enter_context(tc.tile_pool(name="ap", bufs=6))
        e_pool = ctx.enter_context(tc.tile_pool(name="E", bufs=5))
        dpsum = ctx.enter_context(tc.tile_pool(name="dpsum", bufs=4,
                                               space="PSUM"))

        staged = {}

        def issue_stage(g):
            # Stage kd rows [G*g, G*g+n) to partitions 0/1:
            #   sl: p0 = ones,    p1 = m'_mine cols (mTm rows, flattened)
            #   sr: p0 = m'T rows, p1 = -ones
            if g in staged or g >= NG:
                return
            kd0 = G * g
            n = min(G, KD - kd0)
            b, r = divmod(kd0, KDB)
            sl = sl_tiles[g % 2]
            sr = sr_tiles[g % 2]
            nc.sync.dma_start(sl[1:2, 0:n * 128], mTm[b][r:r + n, :])
            nc.sync.dma_start(sr[0:1, 0:n * 1024], mT[b][r:r + n, :])
            staged.pop(g - 2, None)
            staged[g] = (sl, sr)

        def stage_aps(k, slot):
            kd = k * D + slot
            g, o = divmod(kd, G)
            issue_stage(g)
            issue_stage(g + 1)  # prefetch next group into the other buffer
            sl, sr = staged[g]
            return sl[:, o * 128:(o + 1) * 128], sr, o

        def pair_matmul(k, s0, s1, jh):
            # PSUM [128, 2, 512]: diffs for dims s0,s1 on j-half jh
            ps = dpsum.tile([RPC, 2, 512], FP32, tag="pp", name="pp")
            for d, slot in enumerate((s0, s1)):
                lhs_ap, sr, o = stage_aps(k, slot)
                rhs_ap = sr[:, o * 1024 + jh * 512:o * 1024 + jh * 512 + 512]
                nc.tensor.matmul(ps[:, d, :], lhs_ap, rhs_ap,
                                 start=True, stop=True)
            return ps

        def w_matmul(k):
            ps = dpsum.tile([RPC, 2, 512], FP32, tag="pp", name="pw")
            lhs_ap, sr, o = stage_aps(k, 4)
            for jh in range(2):
                rhs_ap = sr[:, o * 1024 + jh * 512:o * 1024 + jh * 512 + 512]
                nc.tensor.matmul(ps[:, jh, :], lhs_ap, rhs_ap,
                                 start=True, stop=True)
            return ps

        for rep in range(reps):
          staged.clear()
          # Per k:  r01 = |d0|+|d1| via DVE reduce-abs-add straight from
          # PSUM (walrus only allows abs on Act activations and on
          # TensorReduce); r23 = |d2|+|d3| via Act flat Abs + Pool merge;
          # |w| split Act/DVE; assembly adds split DVE/Pool; exp on Act.
          pending = []

          def tail(p):
            r01p, r23p, wap, kp = p
            # s = r01 + wa  (reuse r01)
            nc.vector.tensor_tensor(r01p[:, 0:512], r01p[:, 0:512],
                                    wap[:, 0:512], op=ALU.add)
            nc.gpsimd.tensor_tensor(r01p[:, 512:1024], r01p[:, 512:1024],
                                    wap[:, 512:1024], op=ALU.add)
            # l1 = s + r23
            nc.vector.tensor_tensor(r01p[:], r01p[:], r23p[:], op=ALU.add)
            ek = e_pool.tile([RPC, B], FP16)
            nc.scalar.activation(ek, r01p[:], AF.Exp, scale=-1.0,
                                 accum_out=feat[:, kp:kp + 1])

          def emit_A(k, r01, jh):
              pA = pair_matmul(k, 0, 1, jh)
              nc.vector.tensor_reduce(
                  r01[:, jh * 512:(jh + 1) * 512],
                  pA[:].rearrange("p d j -> p j d"),
                  axis=mybir.AxisListType.X, op=ALU.add,
                  apply_absolute_value=True)

          def emit_C(k, r23, jh):
              pC = pair_matmul(k, 2, 3, jh)
              ac = a_pool.tile([RPC, B], FP16, name="ac")
              nc.scalar.activation(
                  ac[:], pC[:].rearrange("p d j -> p (d j)"), AF.Abs)
              nc.gpsimd.tensor_tensor(
                  r23[:, jh * 512:(jh + 1) * 512],
                  ac[:, 0:512], ac[:, 512:1024], op=ALU.add)

          def emit_W(k, wa):
              pW = w_matmul(k)
              nc.scalar.activation(
                  wa[:, 0:512], pW[:, 0, :], AF.Abs)
              nc.vector.tensor_reduce(
                  wa[:, 512:1024],
                  pW[:, 1:2, :].rearrange("p d j -> p j d"),
                  axis=mybir.AxisListType.X, op=ALU.add,
                  apply_absolute_value=True)

          for k in range(K):
            r01 = t_pool.tile([RPC, B], FP16, name="r01")
            r23 = t_pool.tile([RPC, B], FP16, name="r23")
            wa = w_pool.tile([RPC, B], FP16, name="wa")
            for tok in V_EMIT:
                if tok == "A0":
                    emit_A(k, r01, 0)
                elif tok == "A1":
                    emit_A(k, r01, 1)
                elif tok == "C0":
                    emit_C(k, r23, 0)
                elif tok == "C1":
                    emit_C(k, r23, 1)
                elif tok == "W":
                    emit_W(k, wa)
                elif pending:
                    tail(pending.pop(0))
            pending.append((r01, r23, wa, k))
            while len(pending) > V_DEPTH:
                tail(pending.pop(0))
          for p in pending:
            tail(p)

        nc.sync.dma_start(out_d[:, F:F + K], feat[:])

    _hoist_excess_waits(nc)
    return nc


_NC_CACHE = None


def _get_nc():
    global _NC_CACHE
    if _NC_CACHE is None:
        _NC_CACHE = build_nc()
    return _NC_CACHE


def kernel(x: np.ndarray, T: np.ndarray) -> np.ndarray:
    x = np.ascontiguousarray(np.asarray(x, dtype=np.float32))
    T = np.ascontiguousarray(np.asarray(T, dtype=np.float32))
    assert x.shape == (B, F) and T.shape == (F, KD)
    nc = _get_nc()
    in_maps = [
        {"x": x, "t": T, "xm": x[c * RPC:(c + 1) * RPC]}
        for c in range(NCORES)
    ]
    res = run_bass_kernel_spmd(nc, in_maps, list(range(NCORES)))
    return np.concatenate([res.results[c]["out"] for c in range(NCORES)],
                          axis=0)
